# revision 25
# baseline (speedup 1.0000x reference)
"""AdderVDSR kernel for 8 TRN2 NeuronCores.

Mathematical collapse: every AdderNet block computes
    relu(-sum_{c,kh,kw} |patch - w|)
and the inner sum of 576 absolute values of continuous random quantities is
strictly positive, so each block outputs exactly 0 in fp32.  After the first
adder layer the hidden state is identically zero and stays zero, so

    reference(x, ...) == pixel_shuffle(conv3(x, up_w, up_b), 2) + out_b

bit-exactly (conv3 of a zero tensor is exactly zero; relu of a negative
number is exactly 0.0).  The kernel therefore only computes the 3->12 channel
3x3 up-conv, the pixel shuffle, and the two bias adds.

Distribution: data-parallel over H.  Core i computes pre-shuffle rows
[16*i, 16*i+16) -> output rows [32*i, 32*i+32).  The host shards x into
per-core im2col patch tensors in bf16 (layout replication only; all FLOPs
run on device; fp32 PSUM accumulate keeps rel err ~1e-3).  On device: one
bf16 matmul per (parity, batch, psum-bank-quad); biases fold in via a ones
row; the pixel-shuffle column interleave happens in the PSUM->SBUF stage
(stride-2 destinations, VectorE b=0 / ScalarE b=1, overlapped with the
odd-parity matmuls); output stages as bf16 and DMAs out on two HWDGE rings.
"""

import numpy as np

import concourse.bass as bass
import concourse.mybir as mybir
from concourse.bass_utils import run_bass_kernel_spmd

N_CORES = 8
B, C, H, W = 2, 3, 128, 128
RH = H // N_CORES          # 16 pre-shuffle rows per core
NPIX = B * RH * W          # 4096 pre-shuffle pixels per core
OC = 12                    # up-conv output channels (= 4*C)
K = 28                     # im2col contraction: 27 taps + ones row (bias)
XW = NPIX + 16             # xcol width: patches + packed weight columns

_f32 = mybir.dt.float32
_bf16 = mybir.dt.bfloat16


def build_graph():
    nc = bass.Bass()
    xcol = nc.declare_dram_parameter("xcol", [K, XW], _bf16, isOutput=False)
    out = nc.declare_dram_parameter("out", [B, C, 2 * RH, 2 * W], _f32, isOutput=True)

    with (
        nc.sbuf_tensor([K, XW], _bf16) as P,
        nc.sbuf_tensor([38, NPIX], _f32) as sb_out,
        nc.psum_tensor([38, NPIX // 2], _f32) as pse,
        nc.psum_tensor([38, NPIX // 2], _f32) as pso,
        nc.semaphore("dma_in") as dma_in,
        nc.semaphore("dma_in2") as dma_in2,
        nc.semaphore("mm_sem") as mm_sem,
        nc.semaphore("cp0") as cp0,
        nc.semaphore("cp1") as cp1,
        nc.semaphore("dma_out_sem") as dma_out_sem,
        nc.Block() as block,
    ):
        # xcol column layout: [wb (16) | b=0 patches (2048) | b=1 patches (2048)]
        def wslice(dc):
            return P[:, 6 * dc : 6 * dc + 6]

        def rhslice(b, rq):
            lo = 16 + b * (RH * W) + rq * 512
            return P[0:K, lo : lo + 512]

        def out_dma(eng, b, c):
            src = sb_out[32 * b + 2 * c : 32 * b + 2 * c + 2, :].rearrange(
                "dr (r col) -> dr r col", r=RH, col=2 * W
            )
            dst = out[b, c, :, :].rearrange("(r dr) col -> dr r col", dr=2)
            return eng.dma_start(out=dst, in_=src).then_inc(dma_out_sem, 16)

        @block.gpsimd
        def _(gpsimd):
            # Input DMAs from the otherwise-idle Pool engine so descriptor
            # setup overlaps SP/ACT preamble work.  Split so the b=0 matmuls
            # start while the b=1 half is still in flight.
            gpsimd.dma_start(
                out=P[:, : 16 + RH * W], in_=xcol[:, : 16 + RH * W]
            ).then_inc(dma_in, 16)
            gpsimd.dma_start(
                out=P[:, 16 + RH * W :], in_=xcol[:, 16 + RH * W :]
            ).then_inc(dma_in2, 16)

        @block.sync
        def _(sync):
            # b=0 output rows + the last b=1 row; the other two b=1 rows go
            # out on the ACT ring in parallel.
            sync.wait_ge(cp0, 2)
            for c in range(C):
                out_dma(sync, 0, c)
            sync.wait_ge(cp1, 2)
            out_dma(sync, 1, 2)
            sync.wait_ge(dma_out_sem, 96)

        @block.tensor
        def _(tensor):
            # Parity-outer order: all even-channel matmuls (both batches)
            # first, so the even copies overlap the odd matmuls.  pse uses
            # PSUM banks 0-3, pso banks 4-7: no PE-write/engine-read overlap
            # on the same bank.  Partition 32*b + (c*2+dr); slot r*W+col.
            for dc, pst in ((0, pse), (1, pso)):
                for b in range(B):
                    if dc == 0:
                        tensor.wait_ge(dma_in if b == 0 else dma_in2, 16)
                    pb = 32 * b
                    for rq in range(4):
                        o = pst[pb : pb + 6, rq * 512 : (rq + 1) * 512]
                        mm = tensor.matmul(
                            o, lhsT=wslice(dc), rhs=rhslice(b, rq), start=True, stop=True
                        )
                mm.then_inc(mm_sem, 1)

        # PSUM -> SBUF staging with the pixel-shuffle column interleave
        # (stride-2 destinations).  Lane-aligned; VectorE takes b=0 while
        # ScalarE takes b=1.
        @block.vector
        def _(vector):
            vector.wait_ge(mm_sem, 1)
            vector.tensor_copy(sb_out[0:6, 0:NPIX:2], pse[0:6, :]).then_inc(cp0, 1)
            vector.wait_ge(mm_sem, 2)
            vector.tensor_copy(sb_out[0:6, 1:NPIX:2], pso[0:6, :]).then_inc(cp0, 1)

        @block.scalar
        def _(scalar):
            # Dummy tiny copy: pulls the ACT_TABLE_LOAD for Copy forward,
            # off the post-matmul critical path.
            scalar.wait_ge(dma_in, 16)
            scalar.copy(sb_out[32:33, 0:16], P[0:1, 0:16])
            scalar.wait_ge(mm_sem, 1)
            scalar.copy(sb_out[32:38, 0:NPIX:2], pse[32:38, :]).then_inc(cp1, 1)
            scalar.wait_ge(mm_sem, 2)
            scalar.copy(sb_out[32:38, 1:NPIX:2], pso[32:38, :]).then_inc(cp1, 1)
            # Two b=1 output rows on the ACT HWDGE ring (parallel with SP).
            # Self-wait: the DMA must not read sb_out before the deep ACT
            # pipeline has retired the copies.
            scalar.wait_ge(cp1, 2)
            for c in range(2):
                out_dma(scalar, 1, c)

    return nc


def make_in_maps(x, up_w, up_b, out_b):
    """Shard inputs: per-core im2col patches with packed weight columns."""
    import ml_dtypes

    bf16 = ml_dtypes.bfloat16
    x = np.asarray(x, dtype=np.float32)
    up_w = np.asarray(up_w, dtype=np.float32)
    up_b = np.asarray(up_b, dtype=np.float32)
    out_b = np.asarray(out_b, dtype=np.float32)

    # wb[c2*9+kh*3+kw, 6*dc + (c*2+dr)] = up_w[c*4+dr*2+dc, c2, kh, kw]
    # wb[27, 6*dc + (c*2+dr)] = up_b[o] + out_b[c]
    wb = np.zeros((K, 16), dtype=np.float32)
    for c in range(C):
        for dr in range(2):
            for dc in range(2):
                o = c * 4 + dr * 2 + dc
                col = 6 * dc + c * 2 + dr
                wb[:27, col] = up_w[o].reshape(27)
                wb[27, col] = up_b[o] + out_b[c]

    xp = np.zeros((B, C, H + 2, W + 2), dtype=np.float32)
    xp[:, :, 1 : H + 1, 1 : W + 1] = x

    in_maps = []
    for i in range(N_CORES):
        xcol = np.empty((K, XW), dtype=np.float32)
        pat = xcol[:, 16:].reshape(K, B, RH, W)
        for c in range(C):
            for kh in range(3):
                for kw in range(3):
                    k = c * 9 + kh * 3 + kw
                    pat[k] = xp[:, c, 16 * i + kh : 16 * i + kh + RH, kw : kw + W]
        pat[27] = 1.0
        xcol[:, :16] = wb
        in_maps.append({"xcol": xcol.astype(bf16)})
    return in_maps


def kernel(x, up_w, up_b, in_w, in_b, adder_w, out_w, out_b):
    nc = build_graph()
    in_maps = make_in_maps(x, up_w, up_b, out_b)
    res = run_bass_kernel_spmd(nc, in_maps, core_ids=list(range(N_CORES)))
    slabs = [np.asarray(res.results[i]["out"]) for i in range(N_CORES)]
    return np.concatenate(slabs, axis=2).astype(np.float32)
